# revision 15
# baseline (speedup 1.0000x reference)
"""LoRA-injected linear layer on 8 Trainium2 NeuronCores.

Computes y = x @ (W + down @ up)^T + bias for
  x [4, 2048, 4096] f32, W [4096, 4096] f32, down [4096, 16], up [16, 4096].

Sharding: 2 token-groups x 4 out-feature-groups = 8 cores.
Host folds W_eff = W + down @ up once (0.5 GFLOP, ~0.2% of total work) so the
device kernel is a pure GEMM at the bf16 PE roofline.  Each core computes
y_shard [4096 tokens, 1024 out features]:
  - W_eff^T shard (8 MB bf16) is DMA'd chunk-wise on the Activation queue and
    stays SBUF-resident,
  - x^T token tiles stream on the Sync queue (the first tile is chunked per
    K-tile so the PE starts ~2us in),
  - the it-outer matmul order makes each W chunk feed 4 PSUM banks
    back-to-back, so the first-tile accumulation is never W-DMA-gated,
  - bias add is fused into the PSUM->SBUF drain (DVE), y streams out on the
    Pool queue.

Host side does layout-only transforms plus the small W_eff fold so all DMAs
have >=512B contiguous runs.
"""

import numpy as np

import concourse.bass as bass
import concourse.bacc as bacc
import concourse.mybir as mybir
import concourse.tile as tile
from concourse.bass_utils import run_bass_kernel_spmd

# Problem dims (hardcoded per contract).
B, S, IN, OUT, R = 4, 2048, 4096, 4096, 16
NCORES = 8
TG, OG = 2, 4          # token groups x out-feature groups
T = B * S              # 8192 total tokens
TC = T // TG           # 4096 tokens per core
OC = OUT // OG         # 1024 out features per core
P = 128                # partition dim
NT = TC // P           # 32 token tiles per core
NI = IN // P           # 32 contraction tiles
OB = 512               # PSUM-bank-wide output block
NOB = OC // OB         # 2 output blocks per core

F32 = mybir.dt.float32
BF16 = mybir.dt.bfloat16

_CACHE = {}


def _build_nc():
    nc = bacc.Bacc(None, target_bir_lowering=False)

    # DRAM I/O (per-core shards; same program on all 8 cores).
    # xts[tt, i, it, t] = x^T[it*128+i, tt*256+t]: partition-major so a full
    # tile DMA is one 16 KB contiguous run per partition.
    xts_d = nc.declare_dram_parameter("xts", [NT // 2, P, NI, 2 * P], BF16, isOutput=False)
    wt_d = nc.declare_dram_parameter("wt", [NI, P, OC], BF16, isOutput=False)
    bias_d = nc.declare_dram_parameter("biasb", [P, OC], F32, isOutput=False)
    y_d = nc.declare_dram_parameter("y", [TC, OC], F32, isOutput=True)

    with tile.TileContext(nc) as tc:
        with (
            tc.tile_pool(name="wt", bufs=1) as wt_pool,
            tc.tile_pool(name="const", bufs=1) as const_pool,
            tc.tile_pool(name="io", bufs=4) as io_pool,
            tc.tile_pool(name="out", bufs=3) as out_pool,
            tc.tile_pool(name="psum", bufs=2, space="PSUM") as psum_pool,
        ):
            # W_eff^T tiles, resident for the whole kernel (Activation queue).
            # Chunk 0 is split per output block so the first matmul starts
            # as early as possible.
            wts = []
            for i in range(NI):
                t = wt_pool.tile([P, OC], BF16, name=f"wt{i}", tag=f"wt{i}", bufs=1)
                if i == 0:
                    for ob in range(NOB):
                        osl = slice(ob * OB, (ob + 1) * OB)
                        nc.scalar.dma_start(out=t[:, osl], in_=wt_d[0, :, osl])
                else:
                    nc.scalar.dma_start(out=t[:], in_=wt_d[i])
                wts.append(t)
            bias_sb = const_pool.tile([P, OC], F32, name="bias_sb")
            nc.gpsimd.dma_start(out=bias_sb[:], in_=bias_d[:])

            for tt2 in range(NT // 2):
                xts_t = io_pool.tile(
                    [P, NI, 2 * P], BF16, name="xts_t", tag="xts_t", bufs=4
                )
                if tt2 == 0:
                    # Chunk per K-tile so the first matmul isn't gated on a
                    # whole 2 MB tile.
                    for it in range(NI):
                        nc.sync.dma_start(
                            out=xts_t[:, it, :], in_=xts_d[0, :, it, :]
                        )
                else:
                    nc.sync.dma_start(out=xts_t[:], in_=xts_d[tt2])
                # The last double-tile is processed one token-sub at a time so
                # the end-of-kernel tail drains 2 PSUM banks instead of 4.
                sub_groups = [(0, 1)] if tt2 < NT // 2 - 1 else [(0,), (1,)]
                for subs in sub_groups:
                    ps = {}
                    for sub in subs:
                        for ob in range(NOB):
                            ps[sub, ob] = psum_pool.tile(
                                [P, OB], F32, name="ps", tag=f"ps{sub}{ob}", bufs=2
                            )
                    for it in range(NI):
                        for sub in subs:
                            tsl = slice(sub * P, (sub + 1) * P)
                            for ob in range(NOB):
                                osl = slice(ob * OB, (ob + 1) * OB)
                                nc.tensor.matmul(
                                    ps[sub, ob][:],
                                    lhsT=xts_t[:, it, tsl],
                                    rhs=wts[it][:, osl],
                                    start=(it == 0),
                                    stop=(it == NI - 1),
                                )
                    # Per-(sub, ob) drain + store so the tail after the last
                    # matmul is a [128, 512] chain, not a [128, 1024] one.
                    for sub in subs:
                        rsl = slice(
                            (tt2 * 2 + sub) * P, (tt2 * 2 + sub + 1) * P
                        )
                        for ob in range(NOB):
                            osl = slice(ob * OB, (ob + 1) * OB)
                            y_sb = out_pool.tile(
                                [P, OB], F32, name="y_sb", tag=f"y{sub}{ob}", bufs=3
                            )
                            nc.vector.tensor_add(
                                out=y_sb[:], in0=ps[sub, ob][:], in1=bias_sb[:, osl]
                            )
                            store_eng = nc.sync if ob == 0 else nc.gpsimd
                            store_eng.dma_start(out=y_d[rsl, osl], in_=y_sb[:])

    nc.compile()
    return nc


def _shard_inputs(x, old_weight, old_bias, lora_down, lora_up):
    import ml_dtypes

    bf16 = np.dtype(ml_dtypes.bfloat16)
    w_eff = np.asarray(old_weight, np.float32) + (
        np.asarray(lora_down, np.float32) @ np.asarray(lora_up, np.float32)
    )
    wtf = w_eff.T.astype(bf16)                              # [IN, OUT]
    x2 = np.ascontiguousarray(x, dtype=np.float32).reshape(T, IN).astype(bf16)
    bias = np.asarray(old_bias, np.float32)
    # 4 cores share each token group's xts; 2 cores share each wt/bias shard.
    xts_g = []
    for g in range(TG):
        xs = x2[g * TC : (g + 1) * TC]                # [TC, IN] bf16
        # xts[tt2, i, it, u] = xs[tt2*256+u, it*128+i]
        xts_g.append(
            np.ascontiguousarray(
                xs.reshape(NT // 2, 2 * P, NI, P).transpose(0, 3, 2, 1)
            )
        )
    wt_j, bias_j = [], []
    for j in range(OG):
        osl = slice(j * OC, (j + 1) * OC)
        wt_j.append(np.ascontiguousarray(wtf[:, osl]).reshape(NI, P, OC))
        bias_j.append(
            np.ascontiguousarray(np.broadcast_to(bias[osl], (P, OC)))
        )
    in_maps = []
    for c in range(NCORES):
        g, j = divmod(c, OG)
        in_maps.append({"xts": xts_g[g], "wt": wt_j[j], "biasb": bias_j[j]})
    return in_maps


def _get_nc():
    if "nc" not in _CACHE:
        _CACHE["nc"] = _build_nc()
    return _CACHE["nc"]


def _unshard(results):
    y = np.empty((T, OUT), dtype=np.float32)
    for c in range(NCORES):
        g, j = divmod(c, OG)
        y[g * TC : (g + 1) * TC, j * OC : (j + 1) * OC] = results[c]["y"]
    return y.reshape(B, S, OUT)


def _run(inputs, trace=False, trace_cores=None):
    nc = _get_nc()
    in_maps = _shard_inputs(**inputs)
    res = run_bass_kernel_spmd(
        nc,
        in_maps,
        list(range(NCORES)),
        trace=trace,
        trace_cores=trace_cores,
    )
    return _unshard(res.results), res


def kernel(**inputs):
    y, _ = _run(inputs)
    return y
